# revision 46
# baseline (speedup 1.0000x reference)
"""CrossFeatureAttention TRN2 kernel (fp8 DoubleRow, folded projections).

Full inputs -> full output. Sharding: data-parallel over (batch b, half of N1)
across 8 cores; each core computes out[b, h*2048:(h+1)*2048, :].

Math per core (q=2048 rows of x1, x2[b] 4096 rows, C=512), using
associativity to fold the Q/K projections and the residual path:

    Q'  = x1 @ (16 Wq^T Wk) + 16 bq Wk     (fp8 DR; Wqk folded on host.
                                            bk is constant per q-row and
                                            cancels in softmax, so dropped)
    S^T = x2t^T-pairs . Q'                 (fp8 DR)  == 16 * scores^T
    P   = exp(S / (16 sqrt(C)))            (ACT -> fp8)
    rs  = 0.25 * colsum(P^T)               (DR matmul with 0.25-constant lhsT)
    A'  = P @ x2                           (fp8 DR)
    att = (0.25 A') @ Wv^T                 (fp8 DR over the short C axis)
    at8 = att * recip(rs) * 512            (DVE STT -> fp8; == 512*attended,
                                            the 0.25 scales cancel via recip)
    R   = x1 @ (Wo Wq)^T + (bq+bv) Wo^T + bo   (bf16 matmul; residual+output
                                            projection folded on host, bv
                                            exact because rs*recip == 1)
    out = R + at8 @ Wo^T / 512             (fp8 DR + DVE STT)

All fp8 matmuls use MatmulPerfMode.DoubleRow with operands holding
contraction k-tile pairs in [128, 2, F] layout (2 rows/cycle).  Per q-chunk
of 512 rows, the A' accumulation is interleaved into the S loop with a lag;
R fills the early-j slots; the rowsum runs as a prefix sweep + post-flush
tail so its PSUM slot slots into the pA rotation cleanly.
"""

import os
import sys

import numpy as np

for _p in ("/root/.axon_site", "/root/.axon_site/_ro/trn_rl_repo",
           "/root/.axon_site/_ro/pypackages"):
    if _p not in sys.path and os.path.isdir(_p):
        sys.path.append(_p)

import ml_dtypes

import concourse.bacc as bacc
import concourse.mybir as mybir
import concourse.tile as tile
from concourse.bass_utils import run_bass_kernel_spmd

F32 = mybir.dt.float32
BF16 = mybir.dt.bfloat16
F8 = mybir.dt.float8e4
AF = mybir.ActivationFunctionType
ALU = mybir.AluOpType
DR = mybir.MatmulPerfMode.DoubleRow

B, N1, N2, C = 4, 4096, 4096, 512
NCORES = 8
QROWS = N1 * B // NCORES          # 2048 q rows per core
QC = 512                          # q-chunk
NQC = QROWS // QC                 # 4 chunks
KT = N2 // 128                    # 32 k-tiles
CCH = C // 128                    # 4 contraction chunks
SCALE = 1.0 / float(np.sqrt(C))
A_SCALE = 0.25                    # keeps unnormalized A' inside fp8 range
WQK_SCALE = 16.0                  # lifts Wq^T@Wk entries into fp8 range
AT_SCALE = 512.0                  # lifts attended (~0.005 sigma) into fp8

_BUILT = None


def build():
    nc = bacc.Bacc(None, target_bir_lowering=False, debug=False)

    x1t_d = nc.dram_tensor("x1t", [128, CCH, QROWS], BF16, kind="ExternalInput")
    x1t8_d = nc.dram_tensor("x1t8", [128, CCH, QROWS], F8, kind="ExternalInput")
    x2t_d = nc.dram_tensor("x2t", [128, CCH, N2], F8, kind="ExternalInput")
    x2p_d = nc.dram_tensor("x2p", [128, KT, C], F8, kind="ExternalInput")
    wqo_d = nc.dram_tensor("wqo", [128, CCH, C], BF16, kind="ExternalInput")
    wqk_d = nc.dram_tensor("wqk8", [128, CCH, C], F8, kind="ExternalInput")
    wv_d = nc.dram_tensor("wv8", [128, CCH, C], F8, kind="ExternalInput")
    wo_d = nc.dram_tensor("wo8", [128, CCH, C], F8, kind="ExternalInput")
    bqk_d = nc.dram_tensor("bqk", [128, CCH], F32, kind="ExternalInput")
    bct_d = nc.dram_tensor("bct", [C], F32, kind="ExternalInput")
    out_d = nc.dram_tensor("out", [NQC * 4, 128, C], F32, kind="ExternalOutput")

    with tile.TileContext(nc) as tc:
        with tc.tile_pool(name="cst", bufs=1) as cst, \
             tc.tile_pool(name="per", bufs=1) as per, \
             tc.tile_pool(name="sb", bufs=1) as sb, \
             tc.tile_pool(name="ps", bufs=1, space="PSUM") as ps:

            # ---- constants / weights (batched DMAs, needed-first order) ----
            # s16/warm come from memsets so the PE warmup matmuls (which
            # cover the DMA lead-in + clock ramp) depend on no DMA.
            s16 = cst.tile([128, 2, 128], F8, name="s16", tag="s16")
            nc.gpsimd.memset(s16[:], A_SCALE)
            warm = cst.tile([128, 2, 512], F8, name="warm", tag="warm")
            nc.gpsimd.memset(warm[:], 0.0)

            wqk8 = cst.tile([128, CCH, C], F8, name="wqk8", tag="wqk8")
            nc.sync.dma_start(out=wqk8[:], in_=wqk_d[:])
            x1t8 = cst.tile([128, CCH, QROWS], F8, name="x1t8", tag="x1t8")
            nc.sync.dma_start(out=x1t8[:, :, 0:QC], in_=x1t8_d[:, :, 0:QC])
            bqk_b = cst.tile([128, CCH], F32, name="bqkb", tag="bqkb")
            nc.sync.dma_start(out=bqk_b[:], in_=bqk_d[:])
            bqk_t = [bqk_b[:, d:d + 1] for d in range(CCH)]

            x2t = per.tile([128, CCH, N2], F8, name="x2t", tag="x2t")
            x2p = per.tile([128, KT, C], F8, name="x2p", tag="x2p")
            nc.sync.dma_start(out=x2t[:, :, 0:512], in_=x2t_d[:, :, 0:512])
            nc.sync.dma_start(out=x2p[:, 0:4, :], in_=x2p_d[:, 0:4, :])
            nc.sync.dma_start(out=x2t[:, :, 512:1024], in_=x2t_d[:, :, 512:1024])
            nc.sync.dma_start(out=x2p[:, 4:8, :], in_=x2p_d[:, 4:8, :])

            wqo_b = cst.tile([128, CCH, C], BF16, name="wqob", tag="wqob")
            nc.sync.dma_start(out=wqo_b[:], in_=wqo_d[:])
            bc_bc = cst.tile([128, C], F32)
            nc.sync.dma_start(out=bc_bc[:], in_=bct_d[:].unsqueeze(0).broadcast_to([128, C]))
            x1t = cst.tile([128, CCH, QROWS], BF16, name="x1tb", tag="x1tb")
            nc.sync.dma_start(out=x1t[:, :, 0:QC], in_=x1t_d[:, :, 0:QC])

            nc.sync.dma_start(out=x2t[:, :, 1024:2048], in_=x2t_d[:, :, 1024:2048])
            nc.sync.dma_start(out=x2p[:, 8:16, :], in_=x2p_d[:, 8:16, :])
            nc.sync.dma_start(out=x1t8[:, :, QC:QROWS], in_=x1t8_d[:, :, QC:QROWS])
            wv8 = cst.tile([128, CCH, C], F8, name="wv8", tag="wv8")
            nc.sync.dma_start(out=wv8[:], in_=wv_d[:])
            wo8p = cst.tile([128, CCH, C], F8, name="wo8p", tag="wo8p")
            nc.sync.dma_start(out=wo8p[:], in_=wo_d[:])
            nc.sync.dma_start(out=x2t[:, :, 2048:4096], in_=x2t_d[:, :, 2048:4096])
            nc.sync.dma_start(out=x2p[:, 16:KT, :], in_=x2p_d[:, 16:KT, :])
            nc.sync.dma_start(out=x1t[:, :, QC:QROWS], in_=x1t_d[:, :, QC:QROWS])

            # ---- per-chunk state (double buffered across chunks) ----
            def chunk_tiles():
                return {
                    "qp8": sb.tile([128, CCH, QC], F8, name="qp8", tag="qp8", bufs=2),
                    "a8": sb.tile([128, CCH, QC], F8, name="a8", tag="a8", bufs=2),
                    "at8": sb.tile([128, CCH, QC], F8, name="at8", tag="at8", bufs=2),
                    "pt8": [sb.tile([128, 2, QC], F8, name=f"pt{j}", tag=f"pt{j}",
                                    bufs=2) for j in range(KT // 2)],
                    "recip": sb.tile([128, QC], F32, name="recip", tag="recip", bufs=2),
                    "rsb": sb.tile([128, QC // 128, C], F32, name="rsb", tag="rsb",
                                   bufs=2),
                }

            st = [None] * NQC

            def emit_qprime_cch(i, cch, dve_cast=False):
                # Q'^T[c,q] = sum_d Wqk[d,c] x1^T[d,q] + bqk[c]  (fp8 DR)
                if cch == 0:
                    st[i] = chunk_tiles()
                q0 = i * QC
                pp = ps.tile([128, QC], F32, name="qpp", tag="pR", bufs=2)
                for j2 in range(2):
                    nc.tensor.matmul(
                        pp[:],
                        lhsT=wqk8[:, 2 * j2:2 * j2 + 2, cch * 128:(cch + 1) * 128],
                        rhs=x1t8[:, 2 * j2:2 * j2 + 2, q0:q0 + QC],
                        start=(j2 == 0), stop=(j2 == 1), perf_mode=DR)
                if dve_cast:
                    # chunk 0 only: cast on the (idle) DVE so the Scalar
                    # queue starts the exp stream with no backlog
                    nc.vector.tensor_add(
                        out=st[i]["qp8"][:, cch, :], in0=pp[:],
                        in1=bqk_t[cch][:].broadcast_to([128, QC]))
                else:
                    nc.scalar.activation(st[i]["qp8"][:, cch, :], pp[:],
                                         AF.Identity, bias=bqk_t[cch][:])

            def emit_r_rb(i, rb):
                # R[q,c'] = sum_c x1[q,c] Wqo[c',c] + bct  (bf16, residual +
                # output projection of the Q path folded on host)
                q0 = i * QC + rb * 128
                rp = ps.tile([128, C], F32, name="rp", tag="pR", bufs=2)
                for cc in range(CCH):
                    nc.tensor.matmul(rp[:],
                                     lhsT=x1t[:, cc, q0:q0 + 128],
                                     rhs=wqo_b[:, cc, :],
                                     start=(cc == 0), stop=(cc == CCH - 1))
                nc.vector.tensor_add(out=st[i]["rsb"][:, rb, :], in0=rp[:],
                                     in1=bc_bc[:])

            def emit_s_kt(i, kt):
                # S^T[k-tile, q] = sum_c x2^T[c,k] Q'^T[c,q]; exp -> fp8
                sp = ps.tile([128, QC], F32, name="sps", tag="pA", bufs=2)
                for j2 in range(2):
                    nc.tensor.matmul(
                        sp[:],
                        lhsT=x2t[:, 2 * j2:2 * j2 + 2, kt * 128:(kt + 1) * 128],
                        rhs=st[i]["qp8"][:, 2 * j2:2 * j2 + 2, :],
                        start=(j2 == 0), stop=(j2 == 1), perf_mode=DR)
                nc.scalar.activation(st[i]["pt8"][kt // 2][:, kt % 2, :], sp[:],
                                     AF.Exp, scale=float(SCALE / WQK_SCALE))

            def emit_r_j(i, j, rs):
                nc.tensor.matmul(rs[:], lhsT=s16[:], rhs=st[i]["pt8"][j][:],
                                 start=(j == 0), stop=(j == KT // 2 - 1),
                                 perf_mode=DR)

            def emit_ap_j(i, j, ap):
                # A'^T[c,q] += x2[k,c-block]^T-pairs . P^T[k,q]
                for cch in range(CCH):
                    nc.tensor.matmul(ap[cch // 2][:, cch % 2, :],
                                     lhsT=x2p[:, 2 * j:2 * j + 2,
                                              cch * 128:(cch + 1) * 128],
                                     rhs=st[i]["pt8"][j][:],
                                     start=(j == 0), stop=(j == KT // 2 - 1),
                                     perf_mode=DR)

            LAG = 2

            def emit_chunk_core(i, t_prev=None, qp_next=None):
                # S loop with A' groups lagged in behind the exps; the
                # previous chunk's T projection and this chunk's R fill the
                # early-j slots.  The lag flush runs cch-major so each a8
                # cast issues as soon as its accumulator completes; the
                # rowsum runs as a prefix sweep + post-flush tail (rs
                # allocated after the loop's sp tiles so the pA rotation
                # stays clean).
                ap = [ps.tile([128, 2, QC], F32, name="aps", tag="pB", bufs=2)
                      for _ in range(2)]
                for j in range(KT // 2):
                    # extras sit between the two S matmuls so the exp
                    # stream catches up before the next sp slot is claimed
                    emit_s_kt(i, 2 * j)
                    if t_prev is not None and j in (1, 2):
                        emit_t_rb(t_prev, 2 * (j - 1))
                        emit_t_rb(t_prev, 2 * (j - 1) + 1)
                    if 3 <= j <= 6:
                        emit_r_rb(i, j - 3)
                    if qp_next is not None and 7 <= j <= 13 and (j - 7) % 2 == 0:
                        emit_qprime_cch(qp_next, (j - 7) // 2)
                    emit_s_kt(i, 2 * j + 1)
                    if j >= LAG:
                        emit_ap_j(i, j - LAG, ap)
                rs = ps.tile([128, QC], F32, name="rs", tag="pA", bufs=2)
                for j in range(KT // 2 - LAG):
                    emit_r_j(i, j, rs)
                # scaled fp8 copies of A' (x0.25 keeps it inside fp8 range;
                # the scales in rs and a8 cancel through recip) — one wide
                # ACT op per 2-bank accumulator halves the cast overhead
                for h in range(2):
                    for cch in (2 * h, 2 * h + 1):
                        for j in range(KT // 2 - LAG, KT // 2):
                            nc.tensor.matmul(
                                ap[h][:, cch % 2, :],
                                lhsT=x2p[:, 2 * j:2 * j + 2,
                                         cch * 128:(cch + 1) * 128],
                                rhs=st[i]["pt8"][j][:],
                                start=False, stop=(j == KT // 2 - 1),
                                perf_mode=DR)
                    nc.scalar.activation(st[i]["a8"][:, 2 * h:2 * h + 2, :],
                                         ap[h][:], AF.Copy,
                                         scale=float(A_SCALE))
                for j in range(KT // 2 - LAG, KT // 2):
                    emit_r_j(i, j, rs)
                nc.vector.reciprocal_approx_fast(out=st[i]["recip"][:], in_=rs[:])

            def emit_att_at8(i):
                # att^T[d,q] = sum_c Wv^T[c,d] A'^T[c,q];
                # at8 = att * recip * AT_SCALE  (== AT_SCALE * attended)
                for h in range(2):
                    pp = ps.tile([128, 2, QC], F32, name="attp", tag="pB",
                                 bufs=2)
                    for i2 in range(2):
                        d = 2 * h + i2
                        for j2 in range(2):
                            nc.tensor.matmul(
                                pp[:, i2, :],
                                lhsT=wv8[:, 2 * j2:2 * j2 + 2,
                                         d * 128:(d + 1) * 128],
                                rhs=st[i]["a8"][:, 2 * j2:2 * j2 + 2, :],
                                start=(j2 == 0), stop=(j2 == 1), perf_mode=DR)
                    for i2 in range(2):
                        d = 2 * h + i2
                        nc.vector.scalar_tensor_tensor(
                            out=st[i]["at8"][:, d, :], in0=pp[:, i2, :],
                            scalar=float(AT_SCALE), in1=st[i]["recip"][:],
                            op0=ALU.mult, op1=ALU.mult)

            def emit_t_rb(i, rb):
                # out[q,c'] = R[q,c'] + (at8^T-pairs . Wo^T-pairs)/AT_SCALE
                tp = ps.tile([128, C], F32, name="tp", tag="pR", bufs=2)
                for j2 in range(2):
                    nc.tensor.matmul(
                        tp[:],
                        lhsT=st[i]["at8"][:, 2 * j2:2 * j2 + 2,
                                          rb * 128:(rb + 1) * 128],
                        rhs=wo8p[:, 2 * j2:2 * j2 + 2, :],
                        start=(j2 == 0), stop=(j2 == 1), perf_mode=DR)
                ot = sb.tile([128, C], F32, name="ot", tag="ot", bufs=3)
                nc.vector.scalar_tensor_tensor(
                    out=ot[:], in0=tp[:], scalar=float(1.0 / AT_SCALE),
                    in1=st[i]["rsb"][:, rb, :], op0=ALU.mult, op1=ALU.add)
                nc.sync.dma_start(out=out_d[i * 4 + rb, :, :], in_=ot[:])

            # ---- schedule ----
            # PE warmup during the DMA lead-in (clock ramp + covers the
            # wqk8/x1t8 transfer before qprime(0) can start)
            def emit_warm(n):
                for w in range(n):
                    wp = ps.tile([128, QC], F32, name="warmp", tag="pA", bufs=2)
                    nc.tensor.matmul(wp[:], lhsT=s16[:], rhs=warm[:],
                                     start=True, stop=True, perf_mode=DR)

            emit_warm(7)
            for cch in range(CCH):
                emit_qprime_cch(0, cch, dve_cast=True)
            for i in range(NQC):
                emit_chunk_core(i, t_prev=(i - 1 if i > 0 else None),
                                qp_next=(i + 1 if i + 1 < NQC else None))
                emit_att_at8(i)
            for rb in range(QC // 128):
                emit_t_rb(NQC - 1, rb)

    nc.compile()
    return nc


def get_built():
    global _BUILT
    if _BUILT is None:
        _BUILT = build()
    return _BUILT


def _pair_layout(a, dt):
    # [512 (contract), X] -> [128, 4, X]: [p, j, x] = a[j*128+p, x]
    return np.ascontiguousarray(
        a.reshape(CCH, 128, -1).transpose(1, 0, 2)).astype(dt)


def make_in_maps(x1, x2, Wq, bq, Wk, bk, Wv, bv, Wo, bo):
    bf = ml_dtypes.bfloat16
    f8 = ml_dtypes.float8_e4m3
    Wq64 = Wq.astype(np.float64)
    Wk64 = Wk.astype(np.float64)
    Wo64 = Wo.astype(np.float64)
    wqk_mat = WQK_SCALE * (Wq64.T @ Wk64)
    wqk8 = _pair_layout(np.ascontiguousarray(wqk_mat.astype(np.float32)), f8)
    # residual + output projection of the Q path folded: x1 @ (Wo Wq)^T
    wqo_mat = (Wo64 @ Wq64).T
    wqo16 = _pair_layout(np.ascontiguousarray(wqo_mat.astype(np.float32)), bf)
    wv8 = _pair_layout(np.ascontiguousarray(Wv.T), f8)
    wo8 = _pair_layout(np.ascontiguousarray(Wo.T), f8)
    # bv folds into the R bias (rs*recip == 1); bk cancels in softmax
    bqk_vec = WQK_SCALE * (bq.astype(np.float64) @ Wk64)
    bqk32 = np.ascontiguousarray(
        bqk_vec.astype(np.float32).reshape(CCH, 128).T).astype(np.float32)
    bct = (Wo64 @ (bq + bv).astype(np.float64) + bo.astype(np.float64))
    bct32 = bct.astype(np.float32)
    x2t8 = [_pair_layout(np.ascontiguousarray(x2[b].T), f8) for b in range(B)]
    x2p8 = [np.ascontiguousarray(
        x2[b].reshape(KT, 128, C).transpose(1, 0, 2)).astype(f8)
        for b in range(B)]
    in_maps = []
    for cid in range(NCORES):
        b, h = cid // 2, cid % 2
        x1s = x1[b, h * QROWS:(h + 1) * QROWS, :]
        x1sT = np.ascontiguousarray(x1s.T)
        in_maps.append({
            "x1t": _pair_layout(x1sT, bf),
            "x1t8": _pair_layout(x1sT, f8),
            "x2t": x2t8[b], "x2p": x2p8[b],
            "wqo": wqo16, "wqk8": wqk8, "wv8": wv8, "wo8": wo8,
            "bqk": bqk32, "bct": bct32,
        })
    return in_maps


LAST_RESULT = None


def kernel(x1, x2, Wq, bq, Wk, bk, Wv, bv, Wo, bo):
    global LAST_RESULT
    nc = get_built()
    in_maps = make_in_maps(x1, x2, Wq, bq, Wk, bk, Wv, bv, Wo, bo)
    trace = bool(os.environ.get("KERNEL_TRACE"))
    res = run_bass_kernel_spmd(nc, in_maps, core_ids=list(range(NCORES)), trace=trace)
    LAST_RESULT = res
    out = np.empty((B, N1, C), dtype=np.float32)
    for cid in range(NCORES):
        b, h = cid // 2, cid % 2
        out[b, h * QROWS:(h + 1) * QROWS, :] = \
            res.results[cid]["out"].reshape(QROWS, C)
    return out


# revision 47
# speedup vs baseline: 1.0065x; 1.0065x over previous
"""CrossFeatureAttention TRN2 kernel (fp8 DoubleRow, folded projections).

Full inputs -> full output. Sharding: data-parallel over (batch b, half of N1)
across 8 cores; each core computes out[b, h*2048:(h+1)*2048, :].

Math per core (q=2048 rows of x1, x2[b] 4096 rows, C=512), using
associativity to fold the Q/K projections and the residual path:

    Q'  = x1 @ (16 Wq^T Wk) + 16 bq Wk     (fp8 DR; Wqk folded on host.
                                            bk is constant per q-row and
                                            cancels in softmax, so dropped)
    S^T = x2t^T-pairs . Q'                 (fp8 DR)  == 16 * scores^T
    P   = exp(S / (16 sqrt(C)))            (ACT -> fp8)
    rs  = 0.25 * colsum(P^T)               (DR matmul with 0.25-constant lhsT)
    A'  = P @ x2                           (fp8 DR)
    att = (0.25 A') @ Wv^T                 (fp8 DR over the short C axis)
    at8 = att * recip(rs) * 512            (DVE STT -> fp8; == 512*attended,
                                            the 0.25 scales cancel via recip)
    R   = x1 @ (Wo Wq)^T + (bq+bv) Wo^T + bo   (bf16 matmul; residual+output
                                            projection folded on host, bv
                                            exact because rs*recip == 1)
    out = R + at8 @ Wo^T / 512             (fp8 DR + DVE STT)

All fp8 matmuls use MatmulPerfMode.DoubleRow with operands holding
contraction k-tile pairs in [128, 2, F] layout (2 rows/cycle).  Per q-chunk
of 512 rows, the A' accumulation is interleaved into the S loop with a lag;
R fills the early-j slots; the rowsum runs as a prefix sweep + post-flush
tail so its PSUM slot slots into the pA rotation cleanly.
"""

import os
import sys

import numpy as np

for _p in ("/root/.axon_site", "/root/.axon_site/_ro/trn_rl_repo",
           "/root/.axon_site/_ro/pypackages"):
    if _p not in sys.path and os.path.isdir(_p):
        sys.path.append(_p)

import ml_dtypes

import concourse.bacc as bacc
import concourse.mybir as mybir
import concourse.tile as tile
from concourse.bass_utils import run_bass_kernel_spmd

F32 = mybir.dt.float32
BF16 = mybir.dt.bfloat16
F8 = mybir.dt.float8e4
AF = mybir.ActivationFunctionType
ALU = mybir.AluOpType
DR = mybir.MatmulPerfMode.DoubleRow

B, N1, N2, C = 4, 4096, 4096, 512
NCORES = 8
QROWS = N1 * B // NCORES          # 2048 q rows per core
QC = 512                          # q-chunk
NQC = QROWS // QC                 # 4 chunks
KT = N2 // 128                    # 32 k-tiles
CCH = C // 128                    # 4 contraction chunks
SCALE = 1.0 / float(np.sqrt(C))
A_SCALE = 0.25                    # keeps unnormalized A' inside fp8 range
WQK_SCALE = 16.0                  # lifts Wq^T@Wk entries into fp8 range
AT_SCALE = 512.0                  # lifts attended (~0.005 sigma) into fp8

_BUILT = None


def build():
    nc = bacc.Bacc(None, target_bir_lowering=False, debug=False)

    x1t_d = nc.dram_tensor("x1t", [128, CCH, QROWS], BF16, kind="ExternalInput")
    x1t8_d = nc.dram_tensor("x1t8", [128, CCH, QROWS], F8, kind="ExternalInput")
    x2t_d = nc.dram_tensor("x2t", [128, CCH, N2], F8, kind="ExternalInput")
    x2p_d = nc.dram_tensor("x2p", [128, KT, C], F8, kind="ExternalInput")
    wqo_d = nc.dram_tensor("wqo", [128, CCH, C], BF16, kind="ExternalInput")
    wqk_d = nc.dram_tensor("wqk8", [128, CCH, C], F8, kind="ExternalInput")
    wv_d = nc.dram_tensor("wv8", [128, CCH, C], F8, kind="ExternalInput")
    wo_d = nc.dram_tensor("wo8", [128, CCH, C], F8, kind="ExternalInput")
    bqk_d = nc.dram_tensor("bqk", [128, CCH], F32, kind="ExternalInput")
    bct_d = nc.dram_tensor("bct", [C], F32, kind="ExternalInput")
    out_d = nc.dram_tensor("out", [NQC * 4, 128, C], F32, kind="ExternalOutput")

    with tile.TileContext(nc) as tc:
        with tc.tile_pool(name="cst", bufs=1) as cst, \
             tc.tile_pool(name="per", bufs=1) as per, \
             tc.tile_pool(name="sb", bufs=1) as sb, \
             tc.tile_pool(name="ps", bufs=1, space="PSUM") as ps:

            # ---- constants / weights (batched DMAs, needed-first order) ----
            # s16/warm come from memsets so the PE warmup matmuls (which
            # cover the DMA lead-in + clock ramp) depend on no DMA.
            s16 = cst.tile([128, 2, 128], F8, name="s16", tag="s16")
            nc.gpsimd.memset(s16[:], A_SCALE)
            warm = cst.tile([128, 2, 512], F8, name="warm", tag="warm")
            nc.gpsimd.memset(warm[:], 0.0)

            wqk8 = cst.tile([128, CCH, C], F8, name="wqk8", tag="wqk8")
            nc.sync.dma_start(out=wqk8[:], in_=wqk_d[:])
            x1t8 = cst.tile([128, CCH, QROWS], F8, name="x1t8", tag="x1t8")
            nc.sync.dma_start(out=x1t8[:, :, 0:QC], in_=x1t8_d[:, :, 0:QC])
            bqk_b = cst.tile([128, CCH], F32, name="bqkb", tag="bqkb")
            nc.sync.dma_start(out=bqk_b[:], in_=bqk_d[:])
            bqk_t = [bqk_b[:, d:d + 1] for d in range(CCH)]

            x2t = per.tile([128, CCH, N2], F8, name="x2t", tag="x2t")
            x2p = per.tile([128, KT, C], F8, name="x2p", tag="x2p")
            nc.sync.dma_start(out=x2t[:, :, 0:512], in_=x2t_d[:, :, 0:512])
            nc.sync.dma_start(out=x2p[:, 0:4, :], in_=x2p_d[:, 0:4, :])
            nc.sync.dma_start(out=x2t[:, :, 512:1024], in_=x2t_d[:, :, 512:1024])
            nc.sync.dma_start(out=x2p[:, 4:8, :], in_=x2p_d[:, 4:8, :])

            wqo_b = cst.tile([128, CCH, C], BF16, name="wqob", tag="wqob")
            nc.sync.dma_start(out=wqo_b[:], in_=wqo_d[:])
            bc_bc = cst.tile([128, C], F32)
            nc.sync.dma_start(out=bc_bc[:], in_=bct_d[:].unsqueeze(0).broadcast_to([128, C]))
            x1t = cst.tile([128, CCH, QROWS], BF16, name="x1tb", tag="x1tb")
            nc.sync.dma_start(out=x1t[:, :, 0:QC], in_=x1t_d[:, :, 0:QC])

            nc.sync.dma_start(out=x2t[:, :, 1024:2048], in_=x2t_d[:, :, 1024:2048])
            nc.sync.dma_start(out=x2p[:, 8:16, :], in_=x2p_d[:, 8:16, :])
            nc.sync.dma_start(out=x1t8[:, :, QC:QROWS], in_=x1t8_d[:, :, QC:QROWS])
            wv8 = cst.tile([128, CCH, C], F8, name="wv8", tag="wv8")
            nc.sync.dma_start(out=wv8[:], in_=wv_d[:])
            wo8p = cst.tile([128, CCH, C], F8, name="wo8p", tag="wo8p")
            nc.sync.dma_start(out=wo8p[:], in_=wo_d[:])
            nc.sync.dma_start(out=x2t[:, :, 2048:4096], in_=x2t_d[:, :, 2048:4096])
            nc.sync.dma_start(out=x2p[:, 16:KT, :], in_=x2p_d[:, 16:KT, :])
            nc.sync.dma_start(out=x1t[:, :, QC:QROWS], in_=x1t_d[:, :, QC:QROWS])

            # ---- per-chunk state (double buffered across chunks) ----
            def chunk_tiles():
                return {
                    "qp8": sb.tile([128, CCH, QC], F8, name="qp8", tag="qp8", bufs=2),
                    "a8": sb.tile([128, CCH, QC], F8, name="a8", tag="a8", bufs=2),
                    "at8": sb.tile([128, CCH, QC], F8, name="at8", tag="at8", bufs=2),
                    "pt8": [sb.tile([128, 2, QC], F8, name=f"pt{j}", tag=f"pt{j}",
                                    bufs=2) for j in range(KT // 2)],
                    "recip": sb.tile([128, QC], F32, name="recip", tag="recip", bufs=2),
                    "rsb": sb.tile([128, QC // 128, C], F32, name="rsb", tag="rsb",
                                   bufs=2),
                }

            st = [None] * NQC

            def emit_qprime_cch(i, cch, dve_cast=False):
                # Q'^T[c,q] = sum_d Wqk[d,c] x1^T[d,q] + bqk[c]  (fp8 DR)
                if cch == 0:
                    st[i] = chunk_tiles()
                q0 = i * QC
                pp = ps.tile([128, QC], F32, name="qpp", tag="pR", bufs=2)
                for j2 in range(2):
                    nc.tensor.matmul(
                        pp[:],
                        lhsT=wqk8[:, 2 * j2:2 * j2 + 2, cch * 128:(cch + 1) * 128],
                        rhs=x1t8[:, 2 * j2:2 * j2 + 2, q0:q0 + QC],
                        start=(j2 == 0), stop=(j2 == 1), perf_mode=DR)
                if dve_cast:
                    # chunk 0 only: cast on the (idle) DVE so the Scalar
                    # queue starts the exp stream with no backlog
                    nc.vector.tensor_add(
                        out=st[i]["qp8"][:, cch, :], in0=pp[:],
                        in1=bqk_t[cch][:].broadcast_to([128, QC]))
                else:
                    nc.scalar.activation(st[i]["qp8"][:, cch, :], pp[:],
                                         AF.Identity, bias=bqk_t[cch][:])

            def emit_r_rb(i, rb):
                # R[q,c'] = sum_c x1[q,c] Wqo[c',c] + bct  (bf16, residual +
                # output projection of the Q path folded on host)
                q0 = i * QC + rb * 128
                rp = ps.tile([128, C], F32, name="rp", tag="pR", bufs=2)
                for cc in range(CCH):
                    nc.tensor.matmul(rp[:],
                                     lhsT=x1t[:, cc, q0:q0 + 128],
                                     rhs=wqo_b[:, cc, :],
                                     start=(cc == 0), stop=(cc == CCH - 1))
                nc.vector.tensor_add(out=st[i]["rsb"][:, rb, :], in0=rp[:],
                                     in1=bc_bc[:])

            def emit_s_kt(i, kt):
                # S^T[k-tile, q] = sum_c x2^T[c,k] Q'^T[c,q]; exp -> fp8
                sp = ps.tile([128, QC], F32, name="sps", tag="pA", bufs=2)
                for j2 in range(2):
                    nc.tensor.matmul(
                        sp[:],
                        lhsT=x2t[:, 2 * j2:2 * j2 + 2, kt * 128:(kt + 1) * 128],
                        rhs=st[i]["qp8"][:, 2 * j2:2 * j2 + 2, :],
                        start=(j2 == 0), stop=(j2 == 1), perf_mode=DR)
                nc.scalar.activation(st[i]["pt8"][kt // 2][:, kt % 2, :], sp[:],
                                     AF.Exp, scale=float(SCALE / WQK_SCALE))

            def emit_r_j(i, j, rs):
                nc.tensor.matmul(rs[:], lhsT=s16[:], rhs=st[i]["pt8"][j][:],
                                 start=(j == 0), stop=(j == KT // 2 - 1),
                                 perf_mode=DR)

            def emit_ap_j(i, j, ap):
                # A'^T[c,q] += x2[k,c-block]^T-pairs . P^T[k,q]
                for cch in range(CCH):
                    nc.tensor.matmul(ap[cch // 2][:, cch % 2, :],
                                     lhsT=x2p[:, 2 * j:2 * j + 2,
                                              cch * 128:(cch + 1) * 128],
                                     rhs=st[i]["pt8"][j][:],
                                     start=(j == 0), stop=(j == KT // 2 - 1),
                                     perf_mode=DR)

            LAG = 2

            def emit_chunk_core(i, t_prev=None, qp_next=None):
                # S loop with A' groups lagged in behind the exps; the
                # previous chunk's T projection and this chunk's R fill the
                # early-j slots.  The lag flush runs cch-major so each a8
                # cast issues as soon as its accumulator completes; the
                # rowsum runs as a prefix sweep + post-flush tail (rs
                # allocated after the loop's sp tiles so the pA rotation
                # stays clean).
                ap = [ps.tile([128, 2, QC], F32, name="aps", tag="pB", bufs=2)
                      for _ in range(2)]
                for j in range(KT // 2):
                    emit_s_kt(i, 2 * j)
                    emit_s_kt(i, 2 * j + 1)
                    if t_prev is not None and j in (1, 2):
                        emit_t_rb(t_prev, 2 * (j - 1))
                        emit_t_rb(t_prev, 2 * (j - 1) + 1)
                    if 3 <= j <= 6:
                        emit_r_rb(i, j - 3)
                    if qp_next is not None and 7 <= j <= 13 and (j - 7) % 2 == 0:
                        emit_qprime_cch(qp_next, (j - 7) // 2)
                    if j >= LAG:
                        emit_ap_j(i, j - LAG, ap)
                rs = ps.tile([128, QC], F32, name="rs", tag="pA", bufs=2)
                for j in range(KT // 2 - LAG):
                    emit_r_j(i, j, rs)
                # scaled fp8 copies of A' (x0.25 keeps it inside fp8 range;
                # the scales in rs and a8 cancel through recip) — one wide
                # ACT op per 2-bank accumulator halves the cast overhead
                for h in range(2):
                    for cch in (2 * h, 2 * h + 1):
                        for j in range(KT // 2 - LAG, KT // 2):
                            nc.tensor.matmul(
                                ap[h][:, cch % 2, :],
                                lhsT=x2p[:, 2 * j:2 * j + 2,
                                         cch * 128:(cch + 1) * 128],
                                rhs=st[i]["pt8"][j][:],
                                start=False, stop=(j == KT // 2 - 1),
                                perf_mode=DR)
                    nc.scalar.activation(st[i]["a8"][:, 2 * h:2 * h + 2, :],
                                         ap[h][:], AF.Copy,
                                         scale=float(A_SCALE))
                for j in range(KT // 2 - LAG, KT // 2):
                    emit_r_j(i, j, rs)
                nc.vector.reciprocal_approx_fast(out=st[i]["recip"][:], in_=rs[:])

            def emit_att_at8(i):
                # att^T[d,q] = sum_c Wv^T[c,d] A'^T[c,q];
                # at8 = att * recip * AT_SCALE  (== AT_SCALE * attended)
                for h in range(2):
                    pp = ps.tile([128, 2, QC], F32, name="attp", tag="pB",
                                 bufs=2)
                    for i2 in range(2):
                        d = 2 * h + i2
                        for j2 in range(2):
                            nc.tensor.matmul(
                                pp[:, i2, :],
                                lhsT=wv8[:, 2 * j2:2 * j2 + 2,
                                         d * 128:(d + 1) * 128],
                                rhs=st[i]["a8"][:, 2 * j2:2 * j2 + 2, :],
                                start=(j2 == 0), stop=(j2 == 1), perf_mode=DR)
                    for i2 in range(2):
                        d = 2 * h + i2
                        nc.vector.scalar_tensor_tensor(
                            out=st[i]["at8"][:, d, :], in0=pp[:, i2, :],
                            scalar=float(AT_SCALE), in1=st[i]["recip"][:],
                            op0=ALU.mult, op1=ALU.mult)

            def emit_t_rb(i, rb):
                # out[q,c'] = R[q,c'] + (at8^T-pairs . Wo^T-pairs)/AT_SCALE
                tp = ps.tile([128, C], F32, name="tp", tag="pR", bufs=2)
                for j2 in range(2):
                    nc.tensor.matmul(
                        tp[:],
                        lhsT=st[i]["at8"][:, 2 * j2:2 * j2 + 2,
                                          rb * 128:(rb + 1) * 128],
                        rhs=wo8p[:, 2 * j2:2 * j2 + 2, :],
                        start=(j2 == 0), stop=(j2 == 1), perf_mode=DR)
                ot = sb.tile([128, C], F32, name="ot", tag="ot", bufs=3)
                nc.vector.scalar_tensor_tensor(
                    out=ot[:], in0=tp[:], scalar=float(1.0 / AT_SCALE),
                    in1=st[i]["rsb"][:, rb, :], op0=ALU.mult, op1=ALU.add)
                nc.sync.dma_start(out=out_d[i * 4 + rb, :, :], in_=ot[:])

            # ---- schedule ----
            # PE warmup during the DMA lead-in (clock ramp + covers the
            # wqk8/x1t8 transfer before qprime(0) can start)
            def emit_warm(n):
                for w in range(n):
                    wp = ps.tile([128, QC], F32, name="warmp", tag="pA", bufs=2)
                    nc.tensor.matmul(wp[:], lhsT=s16[:], rhs=warm[:],
                                     start=True, stop=True, perf_mode=DR)

            emit_warm(7)
            for cch in range(CCH):
                emit_qprime_cch(0, cch, dve_cast=True)
            for i in range(NQC):
                emit_chunk_core(i, t_prev=(i - 1 if i > 0 else None),
                                qp_next=(i + 1 if i + 1 < NQC else None))
                emit_att_at8(i)
            for rb in range(QC // 128):
                emit_t_rb(NQC - 1, rb)

    nc.compile()
    return nc


def get_built():
    global _BUILT
    if _BUILT is None:
        _BUILT = build()
    return _BUILT


def _pair_layout(a, dt):
    # [512 (contract), X] -> [128, 4, X]: [p, j, x] = a[j*128+p, x]
    return np.ascontiguousarray(
        a.reshape(CCH, 128, -1).transpose(1, 0, 2)).astype(dt)


def make_in_maps(x1, x2, Wq, bq, Wk, bk, Wv, bv, Wo, bo):
    bf = ml_dtypes.bfloat16
    f8 = ml_dtypes.float8_e4m3
    Wq64 = Wq.astype(np.float64)
    Wk64 = Wk.astype(np.float64)
    Wo64 = Wo.astype(np.float64)
    wqk_mat = WQK_SCALE * (Wq64.T @ Wk64)
    wqk8 = _pair_layout(np.ascontiguousarray(wqk_mat.astype(np.float32)), f8)
    # residual + output projection of the Q path folded: x1 @ (Wo Wq)^T
    wqo_mat = (Wo64 @ Wq64).T
    wqo16 = _pair_layout(np.ascontiguousarray(wqo_mat.astype(np.float32)), bf)
    wv8 = _pair_layout(np.ascontiguousarray(Wv.T), f8)
    wo8 = _pair_layout(np.ascontiguousarray(Wo.T), f8)
    # bv folds into the R bias (rs*recip == 1); bk cancels in softmax
    bqk_vec = WQK_SCALE * (bq.astype(np.float64) @ Wk64)
    bqk32 = np.ascontiguousarray(
        bqk_vec.astype(np.float32).reshape(CCH, 128).T).astype(np.float32)
    bct = (Wo64 @ (bq + bv).astype(np.float64) + bo.astype(np.float64))
    bct32 = bct.astype(np.float32)
    x2t8 = [_pair_layout(np.ascontiguousarray(x2[b].T), f8) for b in range(B)]
    x2p8 = [np.ascontiguousarray(
        x2[b].reshape(KT, 128, C).transpose(1, 0, 2)).astype(f8)
        for b in range(B)]
    in_maps = []
    for cid in range(NCORES):
        b, h = cid // 2, cid % 2
        x1s = x1[b, h * QROWS:(h + 1) * QROWS, :]
        x1sT = np.ascontiguousarray(x1s.T)
        in_maps.append({
            "x1t": _pair_layout(x1sT, bf),
            "x1t8": _pair_layout(x1sT, f8),
            "x2t": x2t8[b], "x2p": x2p8[b],
            "wqo": wqo16, "wqk8": wqk8, "wv8": wv8, "wo8": wo8,
            "bqk": bqk32, "bct": bct32,
        })
    return in_maps


LAST_RESULT = None


def kernel(x1, x2, Wq, bq, Wk, bk, Wv, bv, Wo, bo):
    global LAST_RESULT
    nc = get_built()
    in_maps = make_in_maps(x1, x2, Wq, bq, Wk, bk, Wv, bv, Wo, bo)
    trace = bool(os.environ.get("KERNEL_TRACE"))
    res = run_bass_kernel_spmd(nc, in_maps, core_ids=list(range(NCORES)), trace=trace)
    LAST_RESULT = res
    out = np.empty((B, N1, C), dtype=np.float32)
    for cid in range(NCORES):
        b, h = cid // 2, cid % 2
        out[b, h * QROWS:(h + 1) * QROWS, :] = \
            res.results[cid]["out"].reshape(QROWS, C)
    return out


# revision 48
# speedup vs baseline: 1.0077x; 1.0012x over previous
"""CrossFeatureAttention TRN2 kernel (fp8 DoubleRow, folded projections).

Full inputs -> full output. Sharding: data-parallel over (batch b, half of N1)
across 8 cores; each core computes out[b, h*2048:(h+1)*2048, :].

Math per core (q=2048 rows of x1, x2[b] 4096 rows, C=512), using
associativity to fold the Q/K projections and the residual path:

    Q'  = x1 @ (16 Wq^T Wk) + 16 bq Wk     (fp8 DR; Wqk folded on host.
                                            bk is constant per q-row and
                                            cancels in softmax, so dropped)
    S^T = x2t^T-pairs . Q'                 (fp8 DR)  == 16 * scores^T
    P   = exp(S / (16 sqrt(C)))            (ACT -> fp8)
    rs  = 0.25 * colsum(P^T)               (DR matmul with 0.25-constant lhsT)
    A'  = P @ x2                           (fp8 DR)
    att = (0.25 A') @ Wv^T                 (fp8 DR over the short C axis)
    at8 = att * recip(rs) * 512            (DVE STT -> fp8; == 512*attended,
                                            the 0.25 scales cancel via recip)
    R   = x1 @ (Wo Wq)^T + (bq+bv) Wo^T + bo   (bf16 matmul; residual+output
                                            projection folded on host, bv
                                            exact because rs*recip == 1)
    out = R + at8 @ Wo^T / 512             (fp8 DR + DVE STT)

All fp8 matmuls use MatmulPerfMode.DoubleRow with operands holding
contraction k-tile pairs in [128, 2, F] layout (2 rows/cycle).  Per q-chunk
of 512 rows, the A' accumulation is interleaved into the S loop with a lag;
R fills the early-j slots; the rowsum runs as a prefix sweep + post-flush
tail so its PSUM slot slots into the pA rotation cleanly.
"""

import os
import sys

import numpy as np

for _p in ("/root/.axon_site", "/root/.axon_site/_ro/trn_rl_repo",
           "/root/.axon_site/_ro/pypackages"):
    if _p not in sys.path and os.path.isdir(_p):
        sys.path.append(_p)

import ml_dtypes

import concourse.bacc as bacc
import concourse.mybir as mybir
import concourse.tile as tile
from concourse.bass_utils import run_bass_kernel_spmd

F32 = mybir.dt.float32
BF16 = mybir.dt.bfloat16
F8 = mybir.dt.float8e4
AF = mybir.ActivationFunctionType
ALU = mybir.AluOpType
DR = mybir.MatmulPerfMode.DoubleRow

B, N1, N2, C = 4, 4096, 4096, 512
NCORES = 8
QROWS = N1 * B // NCORES          # 2048 q rows per core
QC = 512                          # q-chunk
NQC = QROWS // QC                 # 4 chunks
KT = N2 // 128                    # 32 k-tiles
CCH = C // 128                    # 4 contraction chunks
SCALE = 1.0 / float(np.sqrt(C))
A_SCALE = 0.25                    # keeps unnormalized A' inside fp8 range
WQK_SCALE = 16.0                  # lifts Wq^T@Wk entries into fp8 range
AT_SCALE = 512.0                  # lifts attended (~0.005 sigma) into fp8

_BUILT = None


def build():
    nc = bacc.Bacc(None, target_bir_lowering=False, debug=False)

    x1t_d = nc.dram_tensor("x1t", [128, CCH, QROWS], BF16, kind="ExternalInput")
    x1t8_d = nc.dram_tensor("x1t8", [128, CCH, QROWS], F8, kind="ExternalInput")
    x2t_d = nc.dram_tensor("x2t", [128, CCH, N2], F8, kind="ExternalInput")
    x2p_d = nc.dram_tensor("x2p", [128, KT, C], F8, kind="ExternalInput")
    wqo_d = nc.dram_tensor("wqo", [128, CCH, C], BF16, kind="ExternalInput")
    wqk_d = nc.dram_tensor("wqk8", [128, CCH, C], F8, kind="ExternalInput")
    wv_d = nc.dram_tensor("wv8", [128, CCH, C], F8, kind="ExternalInput")
    wo_d = nc.dram_tensor("wo8", [128, CCH, C], F8, kind="ExternalInput")
    bqk_d = nc.dram_tensor("bqk", [128, CCH], F32, kind="ExternalInput")
    bct_d = nc.dram_tensor("bct", [C], F32, kind="ExternalInput")
    out_d = nc.dram_tensor("out", [NQC * 4, 128, C], F32, kind="ExternalOutput")

    with tile.TileContext(nc) as tc:
        with tc.tile_pool(name="cst", bufs=1) as cst, \
             tc.tile_pool(name="per", bufs=1) as per, \
             tc.tile_pool(name="sb", bufs=1) as sb, \
             tc.tile_pool(name="ps", bufs=1, space="PSUM") as ps:

            # ---- constants / weights (batched DMAs, needed-first order) ----
            # s16/warm come from memsets so the PE warmup matmuls (which
            # cover the DMA lead-in + clock ramp) depend on no DMA.
            s16 = cst.tile([128, 2, 128], F8, name="s16", tag="s16")
            nc.gpsimd.memset(s16[:], A_SCALE)
            warm = cst.tile([128, 2, 512], F8, name="warm", tag="warm")
            nc.gpsimd.memset(warm[:], 0.0)

            wqk8 = cst.tile([128, CCH, C], F8, name="wqk8", tag="wqk8")
            nc.sync.dma_start(out=wqk8[:], in_=wqk_d[:])
            x1t8 = cst.tile([128, CCH, QROWS], F8, name="x1t8", tag="x1t8")
            nc.sync.dma_start(out=x1t8[:, :, 0:QC], in_=x1t8_d[:, :, 0:QC])
            bqk_b = cst.tile([128, CCH], F32, name="bqkb", tag="bqkb")
            nc.sync.dma_start(out=bqk_b[:], in_=bqk_d[:])
            bqk_t = [bqk_b[:, d:d + 1] for d in range(CCH)]

            x2t = per.tile([128, CCH, N2], F8, name="x2t", tag="x2t")
            x2p = per.tile([128, KT, C], F8, name="x2p", tag="x2p")
            nc.sync.dma_start(out=x2t[:, :, 0:512], in_=x2t_d[:, :, 0:512])
            nc.sync.dma_start(out=x2p[:, 0:4, :], in_=x2p_d[:, 0:4, :])
            nc.sync.dma_start(out=x2t[:, :, 512:1024], in_=x2t_d[:, :, 512:1024])
            nc.sync.dma_start(out=x2p[:, 4:8, :], in_=x2p_d[:, 4:8, :])

            wqo_b = cst.tile([128, CCH, C], BF16, name="wqob", tag="wqob")
            nc.sync.dma_start(out=wqo_b[:], in_=wqo_d[:])
            bc_bc = cst.tile([128, C], F32)
            nc.sync.dma_start(out=bc_bc[:], in_=bct_d[:].unsqueeze(0).broadcast_to([128, C]))
            x1t = cst.tile([128, CCH, QROWS], BF16, name="x1tb", tag="x1tb")
            nc.sync.dma_start(out=x1t[:, :, 0:QC], in_=x1t_d[:, :, 0:QC])

            nc.sync.dma_start(out=x2t[:, :, 1024:2048], in_=x2t_d[:, :, 1024:2048])
            nc.sync.dma_start(out=x2p[:, 8:16, :], in_=x2p_d[:, 8:16, :])
            nc.sync.dma_start(out=x1t8[:, :, QC:QROWS], in_=x1t8_d[:, :, QC:QROWS])
            wv8 = cst.tile([128, CCH, C], F8, name="wv8", tag="wv8")
            nc.sync.dma_start(out=wv8[:], in_=wv_d[:])
            wo8p = cst.tile([128, CCH, C], F8, name="wo8p", tag="wo8p")
            nc.sync.dma_start(out=wo8p[:], in_=wo_d[:])
            nc.sync.dma_start(out=x2t[:, :, 2048:4096], in_=x2t_d[:, :, 2048:4096])
            nc.sync.dma_start(out=x2p[:, 16:KT, :], in_=x2p_d[:, 16:KT, :])
            nc.sync.dma_start(out=x1t[:, :, QC:QROWS], in_=x1t_d[:, :, QC:QROWS])

            # ---- per-chunk state (double buffered across chunks) ----
            def chunk_tiles():
                return {
                    "qp8": sb.tile([128, CCH, QC], F8, name="qp8", tag="qp8", bufs=2),
                    "a8": sb.tile([128, CCH, QC], F8, name="a8", tag="a8", bufs=2),
                    "at8": sb.tile([128, CCH, QC], F8, name="at8", tag="at8", bufs=2),
                    "pt8": [sb.tile([128, 2, QC], F8, name=f"pt{j}", tag=f"pt{j}",
                                    bufs=2) for j in range(KT // 2)],
                    "recip": sb.tile([128, QC], F32, name="recip", tag="recip", bufs=2),
                    "rsb": sb.tile([128, QC // 128, C], F32, name="rsb", tag="rsb",
                                   bufs=2),
                }

            st = [None] * NQC

            def emit_qprime_cch(i, cch, dve_cast=False):
                # Q'^T[c,q] = sum_d Wqk[d,c] x1^T[d,q] + bqk[c]  (fp8 DR)
                if cch == 0:
                    st[i] = chunk_tiles()
                q0 = i * QC
                pp = ps.tile([128, QC], F32, name="qpp", tag="pR", bufs=2)
                for j2 in range(2):
                    nc.tensor.matmul(
                        pp[:],
                        lhsT=wqk8[:, 2 * j2:2 * j2 + 2, cch * 128:(cch + 1) * 128],
                        rhs=x1t8[:, 2 * j2:2 * j2 + 2, q0:q0 + QC],
                        start=(j2 == 0), stop=(j2 == 1), perf_mode=DR)
                if dve_cast:
                    # chunk 0 only: cast on the (idle) DVE so the Scalar
                    # queue starts the exp stream with no backlog
                    nc.vector.tensor_add(
                        out=st[i]["qp8"][:, cch, :], in0=pp[:],
                        in1=bqk_t[cch][:].broadcast_to([128, QC]))
                else:
                    nc.scalar.activation(st[i]["qp8"][:, cch, :], pp[:],
                                         AF.Identity, bias=bqk_t[cch][:])

            def emit_r_rb(i, rb):
                # R[q,c'] = sum_c x1[q,c] Wqo[c',c] + bct  (bf16, residual +
                # output projection of the Q path folded on host)
                q0 = i * QC + rb * 128
                rp = ps.tile([128, C], F32, name="rp", tag="pR", bufs=2)
                for cc in range(CCH):
                    nc.tensor.matmul(rp[:],
                                     lhsT=x1t[:, cc, q0:q0 + 128],
                                     rhs=wqo_b[:, cc, :],
                                     start=(cc == 0), stop=(cc == CCH - 1))
                nc.vector.tensor_add(out=st[i]["rsb"][:, rb, :], in0=rp[:],
                                     in1=bc_bc[:])

            def emit_s_kt(i, kt):
                # S^T[k-tile, q] = sum_c x2^T[c,k] Q'^T[c,q]; exp -> fp8
                sp = ps.tile([128, QC], F32, name="sps", tag="pA", bufs=2)
                for j2 in range(2):
                    nc.tensor.matmul(
                        sp[:],
                        lhsT=x2t[:, 2 * j2:2 * j2 + 2, kt * 128:(kt + 1) * 128],
                        rhs=st[i]["qp8"][:, 2 * j2:2 * j2 + 2, :],
                        start=(j2 == 0), stop=(j2 == 1), perf_mode=DR)
                nc.scalar.activation(st[i]["pt8"][kt // 2][:, kt % 2, :], sp[:],
                                     AF.Exp, scale=float(SCALE / WQK_SCALE))

            def emit_r_j(i, j, rs):
                nc.tensor.matmul(rs[:], lhsT=s16[:], rhs=st[i]["pt8"][j][:],
                                 start=(j == 0), stop=(j == KT // 2 - 1),
                                 perf_mode=DR)

            def emit_ap_j(i, j, ap):
                # A'^T[c,q] += x2[k,c-block]^T-pairs . P^T[k,q]
                for cch in range(CCH):
                    nc.tensor.matmul(ap[cch // 2][:, cch % 2, :],
                                     lhsT=x2p[:, 2 * j:2 * j + 2,
                                              cch * 128:(cch + 1) * 128],
                                     rhs=st[i]["pt8"][j][:],
                                     start=(j == 0), stop=(j == KT // 2 - 1),
                                     perf_mode=DR)

            LAG = 2

            def emit_chunk_core(i, t_prev=None, qp_next=None):
                # S loop with A' groups lagged in behind the exps; the
                # previous chunk's T projection and this chunk's R fill the
                # early-j slots.  The lag flush runs cch-major so each a8
                # cast issues as soon as its accumulator completes; the
                # rowsum runs as a prefix sweep + post-flush tail (rs
                # allocated after the loop's sp tiles so the pA rotation
                # stays clean).
                ap = [ps.tile([128, 2, QC], F32, name="aps", tag="pB", bufs=2)
                      for _ in range(2)]
                for j in range(KT // 2):
                    emit_s_kt(i, 2 * j)
                    emit_s_kt(i, 2 * j + 1)
                    if t_prev is not None and j in (1, 2):
                        emit_t_rb(t_prev, 2 * (j - 1))
                        emit_t_rb(t_prev, 2 * (j - 1) + 1)
                    if 3 <= j <= 6:
                        emit_r_rb(i, j - 3)
                    if qp_next is not None and 7 <= j <= 13 and (j - 7) % 2 == 0:
                        emit_qprime_cch(qp_next, (j - 7) // 2)
                    if j >= LAG:
                        emit_ap_j(i, j - LAG, ap)
                rs = ps.tile([128, QC], F32, name="rs", tag="pA", bufs=2)
                for j in range(KT // 2 - LAG):
                    emit_r_j(i, j, rs)
                # scaled fp8 copies of A' (x0.25 keeps it inside fp8 range;
                # the scales in rs and a8 cancel through recip) — one wide
                # ACT op per 2-bank accumulator halves the cast overhead
                for h in range(2):
                    for cch in (2 * h, 2 * h + 1):
                        for j in range(KT // 2 - LAG, KT // 2):
                            nc.tensor.matmul(
                                ap[h][:, cch % 2, :],
                                lhsT=x2p[:, 2 * j:2 * j + 2,
                                         cch * 128:(cch + 1) * 128],
                                rhs=st[i]["pt8"][j][:],
                                start=False, stop=(j == KT // 2 - 1),
                                perf_mode=DR)
                    if h == 0:
                        # h0 on ACT (behind the exp drain), h1 on the idle
                        # DVE so the two 1us casts run in parallel
                        nc.scalar.activation(st[i]["a8"][:, 0:2, :],
                                             ap[0][:], AF.Copy,
                                             scale=float(A_SCALE))
                    else:
                        nc.vector.tensor_scalar_mul(
                            st[i]["a8"][:, 2:4, :], ap[1][:], float(A_SCALE))
                for j in range(KT // 2 - LAG, KT // 2):
                    emit_r_j(i, j, rs)
                nc.vector.reciprocal_approx_fast(out=st[i]["recip"][:], in_=rs[:])

            def emit_att_at8(i):
                # att^T[d,q] = sum_c Wv^T[c,d] A'^T[c,q];
                # at8 = att * recip * AT_SCALE  (== AT_SCALE * attended)
                for h in range(2):
                    pp = ps.tile([128, 2, QC], F32, name="attp", tag="pB",
                                 bufs=2)
                    for i2 in range(2):
                        d = 2 * h + i2
                        for j2 in range(2):
                            nc.tensor.matmul(
                                pp[:, i2, :],
                                lhsT=wv8[:, 2 * j2:2 * j2 + 2,
                                         d * 128:(d + 1) * 128],
                                rhs=st[i]["a8"][:, 2 * j2:2 * j2 + 2, :],
                                start=(j2 == 0), stop=(j2 == 1), perf_mode=DR)
                    for i2 in range(2):
                        d = 2 * h + i2
                        nc.vector.scalar_tensor_tensor(
                            out=st[i]["at8"][:, d, :], in0=pp[:, i2, :],
                            scalar=float(AT_SCALE), in1=st[i]["recip"][:],
                            op0=ALU.mult, op1=ALU.mult)

            def emit_t_rb(i, rb):
                # out[q,c'] = R[q,c'] + (at8^T-pairs . Wo^T-pairs)/AT_SCALE
                tp = ps.tile([128, C], F32, name="tp", tag="pR", bufs=2)
                for j2 in range(2):
                    nc.tensor.matmul(
                        tp[:],
                        lhsT=st[i]["at8"][:, 2 * j2:2 * j2 + 2,
                                          rb * 128:(rb + 1) * 128],
                        rhs=wo8p[:, 2 * j2:2 * j2 + 2, :],
                        start=(j2 == 0), stop=(j2 == 1), perf_mode=DR)
                ot = sb.tile([128, C], F32, name="ot", tag="ot", bufs=3)
                nc.vector.scalar_tensor_tensor(
                    out=ot[:], in0=tp[:], scalar=float(1.0 / AT_SCALE),
                    in1=st[i]["rsb"][:, rb, :], op0=ALU.mult, op1=ALU.add)
                nc.sync.dma_start(out=out_d[i * 4 + rb, :, :], in_=ot[:])

            # ---- schedule ----
            # PE warmup during the DMA lead-in (clock ramp + covers the
            # wqk8/x1t8 transfer before qprime(0) can start)
            def emit_warm(n):
                for w in range(n):
                    wp = ps.tile([128, QC], F32, name="warmp", tag="pA", bufs=2)
                    nc.tensor.matmul(wp[:], lhsT=s16[:], rhs=warm[:],
                                     start=True, stop=True, perf_mode=DR)

            emit_warm(7)
            for cch in range(CCH):
                emit_qprime_cch(0, cch, dve_cast=True)
            for i in range(NQC):
                emit_chunk_core(i, t_prev=(i - 1 if i > 0 else None),
                                qp_next=(i + 1 if i + 1 < NQC else None))
                emit_att_at8(i)
            for rb in range(QC // 128):
                emit_t_rb(NQC - 1, rb)

    nc.compile()
    return nc


def get_built():
    global _BUILT
    if _BUILT is None:
        _BUILT = build()
    return _BUILT


def _pair_layout(a, dt):
    # [512 (contract), X] -> [128, 4, X]: [p, j, x] = a[j*128+p, x]
    return np.ascontiguousarray(
        a.reshape(CCH, 128, -1).transpose(1, 0, 2)).astype(dt)


def make_in_maps(x1, x2, Wq, bq, Wk, bk, Wv, bv, Wo, bo):
    bf = ml_dtypes.bfloat16
    f8 = ml_dtypes.float8_e4m3
    Wq64 = Wq.astype(np.float64)
    Wk64 = Wk.astype(np.float64)
    Wo64 = Wo.astype(np.float64)
    wqk_mat = WQK_SCALE * (Wq64.T @ Wk64)
    wqk8 = _pair_layout(np.ascontiguousarray(wqk_mat.astype(np.float32)), f8)
    # residual + output projection of the Q path folded: x1 @ (Wo Wq)^T
    wqo_mat = (Wo64 @ Wq64).T
    wqo16 = _pair_layout(np.ascontiguousarray(wqo_mat.astype(np.float32)), bf)
    wv8 = _pair_layout(np.ascontiguousarray(Wv.T), f8)
    wo8 = _pair_layout(np.ascontiguousarray(Wo.T), f8)
    # bv folds into the R bias (rs*recip == 1); bk cancels in softmax
    bqk_vec = WQK_SCALE * (bq.astype(np.float64) @ Wk64)
    bqk32 = np.ascontiguousarray(
        bqk_vec.astype(np.float32).reshape(CCH, 128).T).astype(np.float32)
    bct = (Wo64 @ (bq + bv).astype(np.float64) + bo.astype(np.float64))
    bct32 = bct.astype(np.float32)
    x2t8 = [_pair_layout(np.ascontiguousarray(x2[b].T), f8) for b in range(B)]
    x2p8 = [np.ascontiguousarray(
        x2[b].reshape(KT, 128, C).transpose(1, 0, 2)).astype(f8)
        for b in range(B)]
    in_maps = []
    for cid in range(NCORES):
        b, h = cid // 2, cid % 2
        x1s = x1[b, h * QROWS:(h + 1) * QROWS, :]
        x1sT = np.ascontiguousarray(x1s.T)
        in_maps.append({
            "x1t": _pair_layout(x1sT, bf),
            "x1t8": _pair_layout(x1sT, f8),
            "x2t": x2t8[b], "x2p": x2p8[b],
            "wqo": wqo16, "wqk8": wqk8, "wv8": wv8, "wo8": wo8,
            "bqk": bqk32, "bct": bct32,
        })
    return in_maps


LAST_RESULT = None


def kernel(x1, x2, Wq, bq, Wk, bk, Wv, bv, Wo, bo):
    global LAST_RESULT
    nc = get_built()
    in_maps = make_in_maps(x1, x2, Wq, bq, Wk, bk, Wv, bv, Wo, bo)
    trace = bool(os.environ.get("KERNEL_TRACE"))
    res = run_bass_kernel_spmd(nc, in_maps, core_ids=list(range(NCORES)), trace=trace)
    LAST_RESULT = res
    out = np.empty((B, N1, C), dtype=np.float32)
    for cid in range(NCORES):
        b, h = cid // 2, cid % 2
        out[b, h * QROWS:(h + 1) * QROWS, :] = \
            res.results[cid]["out"].reshape(QROWS, C)
    return out
